# revision 2
# baseline (speedup 1.0000x reference)
"""Trainium2 Bass kernel for EnhancedAttention (B=2, T=2048, D=1024, H=16, DH=64).

Sharding: 8 cores = 2 batches x 4 head-groups (4 heads each). No collectives;
each core computes a partial out-projection and the host sums the 4 partials
per batch.

v2 rewrite targeting PE density + engine balance:
  - x preloaded whole; proj matmul groups used as ready filler between
    attention groups so the PE never idles long enough to re-throttle (HAM).
  - S matmuls for the two heads of a pair are emitted interleaved so they
    run concurrently in opposite row-halves of the PE array (K=64 each).
  - ScalarE runs softmax exp only; all DMA triggers live on sync/gpsimd.
  - Normalize: reciprocal_approx_fast on the denominator row + gpsimd
    partition_broadcast + one fused scalar_tensor_tensor (evac*recip).
"""
import os
import sys

for _p in ("/opt/trn_rl_repo", "/root/.axon_site/_ro/trn_rl_repo"):
    if os.path.isdir(_p) and _p not in sys.path:
        sys.path.append(_p)

import ml_dtypes
import numpy as np

import concourse.bass as bass  # noqa: F401
import concourse.tile as tile
from concourse import bacc, mybir
from concourse.bass_utils import run_bass_kernel_spmd

B, T, D = 2, 2048, 1024
H, DH = 16, 64
HPC = 4  # heads per core
NCORES = 8
ROPE_THETA = 10000.0

F32 = mybir.dt.float32
BF16 = mybir.dt.bfloat16

TCH = 512  # t-chunk (q-chunk) size
TC = T // TCH  # 4
DC = D // 128  # 8 contraction chunks
NKT = T // 128  # 16 k-tiles


def _rope_tables():
    inv = 1.0 / (ROPE_THETA ** (np.arange(0, DH, 2, dtype=np.float64) / DH))
    f = np.arange(T, dtype=np.float64)[:, None] * inv[None, :]  # [T, 32]
    cos = np.cos(f).T.astype(ml_dtypes.bfloat16)  # [32, T]
    sin = np.sin(f).T.astype(ml_dtypes.bfloat16)
    cs1 = np.ascontiguousarray(np.tile(cos, (4, 1)))  # [128, T]
    # pre-swapped sign pattern: after the strip-swap DMA this lands as
    # [-sin, sin, -sin, sin] against the un-swapped operand
    cs2 = np.ascontiguousarray(np.concatenate([sin, -sin, sin, -sin], axis=0))
    return cs1, cs2


def _build():
    nc = bacc.Bacc("TRN2", target_bir_lowering=False, debug=False, num_devices=NCORES)
    xT_d = nc.dram_tensor("xT", [D, T], BF16, kind="ExternalInput")
    wq_d = nc.dram_tensor("wq", [D, HPC * DH], BF16, kind="ExternalInput")
    wk_d = nc.dram_tensor("wk", [D, HPC * DH], BF16, kind="ExternalInput")
    wv_d = nc.dram_tensor("wv", [D, HPC * DH], BF16, kind="ExternalInput")
    wo_d = nc.dram_tensor("wo", [HPC * DH, D], BF16, kind="ExternalInput")
    y_d = nc.dram_tensor("y", [T, D], F32, kind="ExternalOutput")

    cs1_np, cs2_np = _rope_tables()
    cs1_d = nc.inline_tensor(cs1_np, "cs1")
    cs2_d = nc.inline_tensor(cs2_np, "cs2")

    EXP = mybir.ActivationFunctionType.Exp
    MULT = mybir.AluOpType.mult

    import contextlib
    with tile.TileContext(nc) as tc:
        with (
            contextlib.ExitStack() as _ctx,
            tc.tile_pool(name="sb", bufs=1) as sb,
            tc.tile_pool(name="ropep", bufs=2) as ropep,
            tc.tile_pool(name="ptp", bufs=6) as ptp,
            tc.tile_pool(name="misp", bufs=2) as misp,
            tc.tile_pool(name="ysbp", bufs=3) as ysbp,
        ):
            wq = sb.tile([128, DC, HPC * DH], BF16)
            wk = sb.tile([128, DC, HPC * DH], BF16)
            wv = sb.tile([128, DC, HPC * DH], BF16)
            wo = sb.tile([128, 2, D], BF16)
            cs1 = sb.tile([128, T], BF16)
            cs2 = sb.tile([128, T], BF16)
            xtc = [
                sb.tile([128, DC, TCH], BF16, tag=f"xt{c}", name=f"xt{c}")
                for c in range(TC)
            ]  # x^T split per t-chunk so subtile deps stay clean
            qt = [sb.tile([128, T], BF16, tag=f"qt{p}", name=f"qt{p}") for p in range(2)]
            ktt = [sb.tile([128, T], BF16, tag=f"kt{p}", name=f"kt{p}") for p in range(2)]
            vaug = sb.tile([128, NKT, HPC, DH + 1], BF16)
            ot = [sb.tile([128, T], BF16, tag=f"ot{p}", name=f"ot{p}") for p in range(2)]

            # input DMAs staged by first-need across the three DMA-capable
            # queues (each dma_start occupies its queue for the transfer).
            # xt chunks 2-3 and wo are emitted later (after chunk-0 work).
            xT_r = xT_d.ap().rearrange("(c p) t -> p c t", p=128)
            nc.sync.dma_start(wq[:], wq_d.ap().rearrange("(c p) n -> p c n", p=128))
            nc.scalar.dma_start(xtc[0][:, 4:8, :], xT_r[:, 4:8, 0:TCH])
            nc.gpsimd.dma_start(wk[:], wk_d.ap().rearrange("(c p) n -> p c n", p=128))
            nc.sync.dma_start(xtc[0][:, 0:4, :], xT_r[:, 0:4, 0:TCH])
            nc.scalar.dma_start(xtc[1][:, 0:4, :], xT_r[:, 0:4, TCH : 2 * TCH])
            nc.gpsimd.dma_start(cs2[:], cs2_d.ap())
            nc.sync.dma_start(cs1[:], cs1_d.ap())
            nc.scalar.dma_start(xtc[1][:, 4:8, :], xT_r[:, 4:8, TCH : 2 * TCH])
            nc.gpsimd.dma_start(wv[:], wv_d.ap().rearrange("(c p) n -> p c n", p=128))
            nc.vector.memset(vaug[:, :, :, DH : DH + 1], 1.0)

            # PSUM: s0/s1 2 banks each, o0/o1 1 bank each, pj 2 banks = 8
            sps = _ctx.enter_context(tc.tile_pool(name="sps", bufs=1, space="PSUM"))
            ops = _ctx.enter_context(tc.tile_pool(name="ops", bufs=1, space="PSUM"))
            pjps = _ctx.enter_context(tc.tile_pool(name="pjps", bufs=2, space="PSUM"))

            # PE warm-up to bridge the input-DMA wait
            warm = sb.tile([128, TCH], BF16, name="warm")
            nc.vector.memset(warm, 0.0)
            wps = pjps.tile([128, TCH], F32, tag="pj", name="wps")
            for wi in range(10):
                nc.tensor.matmul(
                    wps[:], warm[:, 0:128], warm[:],
                    start=(wi == 0), stop=(wi == 9),
                )

            _dma_ring = [nc.sync, nc.gpsimd]
            _ring_i = [0]

            def ring():
                _ring_i[0] ^= 1
                return _dma_ring[_ring_i[0]]

            def proj_units(tci):
                """Generator: one yield per PE matmul group of chunk tci."""
                tsl = slice(tci * TCH, (tci + 1) * TCH)
                pend = [None]

                def flush():
                    if pend[0] is not None:
                        pend[0]()
                        pend[0] = None

                for w_sb, dest in ((wq, qt), (wk, ktt)):
                    for p in range(2):
                        ps = pjps.tile([128, TCH], F32, tag="pj", name=f"pj{tci}_{p}")
                        for dc in range(DC):
                            nc.tensor.matmul(
                                ps[:],
                                w_sb[:, dc, p * 128 : (p + 1) * 128],
                                xtc[tci][:, dc, :],
                                start=(dc == 0),
                                stop=(dc == DC - 1),
                            )
                        t1 = ropep.tile([128, TCH], BF16, tag="t1", name=f"t1_{tci}")
                        t2 = ropep.tile([128, TCH], BF16, tag="t2", name=f"t2_{tci}")
                        swt = ropep.tile([128, TCH], BF16, tag="swt", name=f"sws{tci}")
                        nc.vector.tensor_mul(t1[:], ps[:], cs1[:, tsl])
                        nc.vector.tensor_mul(t2[:], ps[:], cs2[:, tsl])
                        flush()
                        for s in range(4):
                            nc.sync.dma_start(
                                swt[s * 32 : (s + 1) * 32, :],
                                t2[(s ^ 1) * 32 : ((s ^ 1) + 1) * 32, :],
                            )

                        def _mkadd(dst, a, b):
                            return lambda: nc.vector.tensor_add(dst, a, b)

                        pend[0] = _mkadd(dest[p][:, tsl], t1[:], swt[:])
                        yield
                for tt in range(4):
                    gt = tci * 4 + tt  # global t-tile / k-tile index
                    ps = pjps.tile([128, TCH], F32, tag="pj", name=f"pjv{gt}")
                    for dc in range(DC):
                        nc.tensor.matmul(
                            ps[:, : HPC * DH],
                            xtc[tci][:, dc, tt * 128 : (tt + 1) * 128],
                            wv[:, dc, :],
                            start=(dc == 0),
                            stop=(dc == DC - 1),
                        )
                    flush()
                    nc.vector.tensor_copy(
                        vaug[:, gt, :, 0:DH],
                        ps[:, : HPC * DH].rearrange("p (h d) -> p h d", h=HPC),
                    )
                    yield
                flush()

            COPYF = mybir.ActivationFunctionType.Copy

            def outproj_units(qc, tail=False):
                for tt in range(4):
                    gtt = qc * 4 + tt
                    for ni in range(2):
                        ypsum = pjps.tile([128, TCH], F32, tag="pj", name=f"y{gtt}_{ni}")
                        for p2 in range(2):
                            nc.tensor.matmul(
                                ypsum[:],
                                ot[p2][:, gtt * 128 : (gtt + 1) * 128],
                                wo[:, p2, ni * TCH : (ni + 1) * TCH],
                                start=(p2 == 0),
                                stop=(p2 == 1),
                            )
                        ysb = ysbp.tile([128, TCH], F32, tag="ysb", name=f"ys{gtt}_{ni}")
                        if tail and ni % 2 == 1:
                            nc.scalar.activation(ysb[:], ypsum[:], COPYF)
                        else:
                            nc.vector.tensor_copy(ysb[:], ypsum[:])
                        yring = (
                            nc.scalar if tail else (nc.gpsimd, nc.sync)[(gtt + ni) % 2]
                        )
                        yring.dma_start(
                            y_d.ap()[
                                gtt * 128 : (gtt + 1) * 128,
                                ni * TCH : (ni + 1) * TCH,
                            ],
                            ysb[:],
                        )
                        yield

            def attn_chunk(qc, filler):
                qsl = slice(qc * TCH, (qc + 1) * TCH)
                nkt = 4 * qc + 4  # causal: k-tiles 0..4qc+3
                for hp in range(2):
                    h0, h1 = 2 * hp, 2 * hp + 1
                    opsums = {
                        h: ops.tile(
                            [128, TCH], F32, tag=f"o{h % 2}", name=f"o{qc}_{h}"
                        )
                        for h in (h0, h1)
                    }
                    prev = None  # (kb, {h: pt tile})
                    for kb in range(0, nkt, 2):
                        next(filler, None)
                        spts = {}
                        pts = {}
                        for h in (h0, h1):
                            spts[h] = sps.tile(
                                [128, 2, TCH], F32, tag=f"s{h % 2}",
                                name=f"s{qc}_{h}_{kb}",
                            )
                            pts[h] = ptp.tile(
                                [128, 2, TCH], BF16, tag="pt",
                                name=f"pt{qc}_{h}_{kb}",
                            )
                        # S matmuls interleaved across the head pair so they
                        # run concurrently in opposite PE row-halves (K=64)
                        for j in range(2):
                            ktile = kb + j
                            off = max(0, 128 * ktile - 512 * qc)
                            for h in (h0, h1):
                                par = h % 2
                                nc.tensor.matmul(
                                    spts[h][:, j, off:],
                                    ktt[hp][
                                        par * 64 : par * 64 + 64,
                                        ktile * 128 : (ktile + 1) * 128,
                                    ],
                                    qt[hp][par * 64 : par * 64 + 64, qsl][:, off:],
                                    start=True,
                                    stop=True,
                                )
                        off0 = max(0, 128 * kb - 512 * qc)  # flat live prefix
                        for h in (h0, h1):
                            nc.scalar.activation(
                                pts[h].rearrange("p a b -> p (a b)")[:, off0:],
                                spts[h].rearrange("p a b -> p (a b)")[:, off0:],
                                EXP,
                                bias=0.0,
                                scale=0.125,
                            )
                        for j in range(2):
                            ktile = kb + j
                            if ktile >= 4 * qc:  # diagonal: mask k > q
                                off = max(0, 128 * ktile - 512 * qc)
                                for h in (h0, h1):
                                    nc.gpsimd.affine_select(
                                        out=pts[h][:, j, off:],
                                        in_=pts[h][:, j, off:],
                                        compare_op=mybir.AluOpType.is_ge,
                                        fill=0.0,
                                        base=0,
                                        pattern=[[1, TCH - off]],
                                        channel_multiplier=-1,
                                    )
                        if prev is not None:
                            pkb, ppts = prev
                            for j in range(2):
                                ktile = pkb + j
                                off = max(0, 128 * ktile - 512 * qc)
                                for h in (h0, h1):
                                    nc.tensor.matmul(
                                        opsums[h][0 : DH + 1, off:],
                                        vaug[:, ktile, h, :],
                                        ppts[h][:, j, off:],
                                        start=(ktile == 0),
                                        stop=(ktile == nkt - 1),
                                    )
                        prev = (kb, pts)
                    # final O' group
                    pkb, ppts = prev
                    for j in range(2):
                        ktile = pkb + j
                        off = max(0, 128 * ktile - 512 * qc)
                        for h in (h0, h1):
                            nc.tensor.matmul(
                                opsums[h][0 : DH + 1, off:],
                                vaug[:, ktile, h, :],
                                ppts[h][:, j, off:],
                                start=(ktile == 0),
                                stop=(ktile == nkt - 1),
                            )
                    next(filler, None)
                    # normalize (baseline-style): evac O'+denom, DMA the denom
                    # row to partition 0, approx-recip, broadcast, multiply
                    for h in (h0, h1):
                        par = h % 2
                        opsum = opsums[h]
                        oraw = misp.tile(
                            [DH + 1, TCH], F32, tag="oraw", name=f"or{qc}_{h}"
                        )
                        nc.vector.tensor_copy(oraw[:], opsum[0 : DH + 1, :])
                        rec0 = misp.tile([1, TCH], F32, tag="rec0", name=f"rc{qc}_{h}")
                        nc.gpsimd.dma_start(rec0[:], oraw[DH : DH + 1, :])
                        nc.vector.reciprocal_approx_fast(rec0[:], rec0[:])
                        bc = misp.tile([64, TCH], F32, tag="bc", name=f"bc{qc}_{h}")
                        nc.gpsimd.partition_broadcast(bc[:], rec0[:])
                        if par == 0:
                            nc.vector.tensor_mul(
                                ot[hp][0:64, qsl], oraw[0:64, :], bc[:]
                            )
                        else:
                            tmpo = misp.tile(
                                [64, TCH], BF16, tag="tmpo", name=f"tp{qc}_{h}"
                            )
                            nc.vector.tensor_mul(tmpo[:], oraw[0:64, :], bc[:])
                            nc.gpsimd.dma_start(ot[hp][64:128, qsl], tmpo[:])

            import itertools

            def interleave(agen, filler):
                # drive attention generator to completion, then drain filler
                for _ in agen:
                    pass
                for _ in filler:
                    pass

            def attn_gen(qc, filler):
                attn_chunk(qc, filler)
                yield

            # emission schedule: P0 eager; A(qc) with P(qc+1) as filler; tail
            for _ in proj_units(0):
                pass
            f1 = proj_units(1)
            attn_chunk(0, f1)
            for _ in f1:
                pass
            nc.gpsimd.dma_start(xtc[2][:, 0:4, :], xT_r[:, 0:4, 2 * TCH : 3 * TCH])
            nc.gpsimd.dma_start(xtc[2][:, 4:8, :], xT_r[:, 4:8, 2 * TCH : 3 * TCH])
            nc.gpsimd.dma_start(wo[:], wo_d.ap().rearrange("(c p) n -> p c n", p=128))
            nc.sync.dma_start(xtc[3][:, 0:4, :], xT_r[:, 0:4, 3 * TCH : 4 * TCH])
            nc.sync.dma_start(xtc[3][:, 4:8, :], xT_r[:, 4:8, 3 * TCH : 4 * TCH])
            op0 = outproj_units(0)
            f2 = itertools.chain(proj_units(2), op0)
            attn_chunk(1, f2)
            for _ in f2:
                pass
            op1 = outproj_units(1)
            f3 = itertools.chain(proj_units(3), op1)
            attn_chunk(2, f3)
            for _ in f3:
                pass
            op2 = outproj_units(2)
            attn_chunk(3, op2)
            for _ in op2:
                pass
            for _ in outproj_units(3, tail=True):
                pass
    nc.compile()
    return nc


_NC_CACHE = []


def _get_nc():
    if not _NC_CACHE:
        _NC_CACHE.append(_build())
    return _NC_CACHE[0]


_LAST_RESULTS = []  # stashed BassKernelResults for test harness introspection


def kernel(x, Wqkv, Wout, _trace=False, **_trace_kwargs):
    x = np.asarray(x, dtype=np.float32)
    Wqkv = np.asarray(Wqkv, dtype=np.float32)
    Wout = np.asarray(Wout, dtype=np.float32)

    nc = _get_nc()
    in_maps = []
    for c in range(NCORES):
        b, g = divmod(c, HPC)
        cols = slice(g * HPC * DH, (g + 1) * HPC * DH)
        rows = slice(g * HPC * DH, (g + 1) * HPC * DH)
        bf = ml_dtypes.bfloat16
        in_maps.append(
            {
                "xT": np.ascontiguousarray(x[b].T.astype(bf)),
                "wq": np.ascontiguousarray(Wqkv[:, 0:D][:, cols].astype(bf)),
                "wk": np.ascontiguousarray(Wqkv[:, D : 2 * D][:, cols].astype(bf)),
                "wv": np.ascontiguousarray(Wqkv[:, 2 * D : 3 * D][:, cols].astype(bf)),
                "wo": np.ascontiguousarray(Wout[rows, :].astype(bf)),
            }
        )

    res = run_bass_kernel_spmd(
        nc, in_maps, core_ids=list(range(NCORES)), trace=_trace, **_trace_kwargs
    )
    _LAST_RESULTS.clear()
    _LAST_RESULTS.append(res)

    out = np.zeros((B, T, D), dtype=np.float32)
    for c in range(NCORES):
        b = c // HPC
        out[b] += res.results[c]["y"]
    return out
